# revision 2
# baseline (speedup 1.0000x reference)
"""Trainium2 Bass kernel for the 3-layer single-step LSTM stack + 2 FC layers.

Reference computation (per batch b, per independent column t of 21):
    xt = x[b, :, t]                       # (128,)
    h1 = sig(o1) * tanh(sig(i1) * tanh(g1)),  gates1 = W1 @ xt + b1   (f-gate unused: c0 = 0)
    h2 = likewise from h1 (H=256), h3 likewise (H=128)
    y  = fc1_w @ h3 + fc1_b               # scalar
    out[b, 0, :] = fc2_w @ y[b, :] + fc2_b

Strategy: pure data-parallel over 8 cores (1024 batches each).

v2: all matmuls in bf16 (1 cycle/row on the PE instead of 4 for fp32);
x is cast+transposed to [I, B, T] bf16 on the host so each DMA line is
2688 contiguous bytes per partition. Activations write bf16 so the DVE
elementwise muls run in 2x_1p mode. The tanh(c) eval can optionally be
replaced by a fitted cubic on the DVE (c is bounded: |c| < 1) to offload
the ACT engine, which is the post-matmul bottleneck.
"""

import sys

if "/opt/trn_rl_repo" not in sys.path:
    sys.path.insert(0, "/opt/trn_rl_repo")

import numpy as np

# Problem constants (hardcoded per contract)
B, I, T = 8192, 128, 21
H1, H2, H3 = 256, 256, 128
NCORES = 8
BS = B // NCORES            # 1024 batches per core

# Tiling
NB = 64                     # batches per tile
NR = NB * T                 # 1344 rows per tile
NMM = 448                   # rows per layer-matmul (3 per tile, bank-aligned psum)
NFC = 336                   # rows per fc chunk (16 batches * 21), 4 per tile

# tanh(x) ~= x*(TP_A + TP_B*x^2) minimax fit on [-1, 1] (c = sig(i)*tanh(g)
# is bounded to (-1,1); typical |c| < 0.5 for this data distribution)
USE_TANH_POLY = False
TP_A = 0.98773837
TP_B = -0.24266598

_prog_cache = {}


def _build_program(n_batch):
    import concourse.bass as bass
    import concourse.tile as tile
    from concourse import mybir

    f32 = mybir.dt.float32
    bf16 = mybir.dt.bfloat16
    AF = mybir.ActivationFunctionType
    n_tiles = n_batch // NB

    nc = bass.Bass()

    # -------- DRAM I/O --------
    x_d = nc.dram_tensor("x", [I, n_batch, T], bf16, kind="ExternalInput")
    w1t_d = nc.dram_tensor("w1t", [128, 768], bf16, kind="ExternalInput")
    w2t_d = nc.dram_tensor("w2t", [256, 768], bf16, kind="ExternalInput")
    w3t_d = nc.dram_tensor("w3t", [256, 384], bf16, kind="ExternalInput")
    b1_d = nc.dram_tensor("b1", [128, 6], f32, kind="ExternalInput")
    b2_d = nc.dram_tensor("b2", [128, 6], f32, kind="ExternalInput")
    b3_d = nc.dram_tensor("b3", [128, 3], f32, kind="ExternalInput")
    f_d = nc.dram_tensor("f", [128, 21], bf16, kind="ExternalInput")
    w2rep_d = nc.dram_tensor("w2rep", [21, NFC], f32, kind="ExternalInput")
    cst_d = nc.dram_tensor("cst", [21, 1], f32, kind="ExternalInput")
    out_d = nc.dram_tensor("out", [21, n_batch], f32, kind="ExternalOutput")

    with tile.TileContext(nc) as tc:
        with (
            tc.tile_pool(name="const", bufs=1) as cpool,
            tc.tile_pool(name="xin", bufs=3) as xpool,
            tc.tile_pool(name="act", bufs=2) as apool,
            tc.tile_pool(name="hbuf", bufs=4) as hpool,
            tc.tile_pool(name="fc2", bufs=2) as fpool,
            tc.tile_pool(name="gates", bufs=2, space=bass.MemorySpace.PSUM) as gpool,
            tc.tile_pool(name="ypsum", bufs=2, space=bass.MemorySpace.PSUM) as ypool,
        ):
            # -------- one-time constant loads --------
            w1t = cpool.tile([128, 768], bf16)
            w2t_k0 = cpool.tile([128, 768], bf16, tag="w2k0")
            w2t_k1 = cpool.tile([128, 768], bf16, tag="w2k1")
            w3t_k0 = cpool.tile([128, 384], bf16, tag="w3k0")
            w3t_k1 = cpool.tile([128, 384], bf16, tag="w3k1")
            b1 = cpool.tile([128, 6], f32, tag="b1")
            b2 = cpool.tile([128, 6], f32, tag="b2")
            b3 = cpool.tile([128, 3], f32, tag="b3")
            fmat = cpool.tile([128, 21], bf16, tag="fmat")
            w2rep = cpool.tile([21, NFC], f32, tag="w2rep")
            cst = cpool.tile([21, 1], f32, tag="cst")
            out_t = cpool.tile([21, n_batch], f32, tag="out_t")
            out_f = cpool.tile([21, n_batch], f32, tag="out_f")

            nc.sync.dma_start(w1t[:], w1t_d[:])
            nc.sync.dma_start(w2t_k0[:], w2t_d[0:128, :])
            nc.sync.dma_start(w2t_k1[:], w2t_d[128:256, :])
            nc.sync.dma_start(w3t_k0[:], w3t_d[0:128, :])
            nc.sync.dma_start(w3t_k1[:], w3t_d[128:256, :])
            nc.sync.dma_start(b1[:], b1_d[:])
            nc.sync.dma_start(b2[:], b2_d[:])
            nc.sync.dma_start(b3[:], b3_d[:])
            nc.sync.dma_start(fmat[:], f_d[:])
            nc.sync.dma_start(w2rep[:], w2rep_d[:])
            nc.sync.dma_start(cst[:], cst_d[:])

            def lstm_layer(rhs_chunks, wchunks, bias, nchunks_h, funcs=None):
                """One LSTM cell step on a row-tile.

                rhs_chunks: list of [128, NR]-view SBUF APs (k-chunks of input)
                wchunks: list of (lhsT tile, col offset) per k-chunk; each holds
                         3*nchunks_h*128 columns laid out i|g|o.
                bias: [128, 3*nchunks_h] SBUF tile
                Returns h chunk tiles (list of nchunks_h [128, NR] tiles).
                """
                nch = 3 * nchunks_h  # total m-chunks (i, g, o)
                sig_i, tg, sig_o = [], [], []
                for c in range(nch):
                    role = c // nchunks_h  # 0=i, 1=g, 2=o
                    ps = gpool.tile([128, 3, 512], f32, tag="gates")
                    for ki, rk in enumerate(rhs_chunks):
                        lhsT = wchunks[ki][:, c * 128:(c + 1) * 128]
                        first = ki == 0
                        last = ki == len(rhs_chunks) - 1
                        for p in range(3):
                            nc.tensor.matmul(
                                ps[:, p, 0:NMM],
                                lhsT,
                                rk[:, p * NMM:(p + 1) * NMM],
                                start=first,
                                stop=last,
                            )
                    dst = apool.tile([128, NR], bf16,
                                     tag=("sig_i", "tg", "sig_o")[role])
                    func = AF.Tanh if role == 1 else AF.Sigmoid
                    nc.scalar.activation(
                        dst[:].rearrange("p (a b) -> p a b", a=3),
                        ps[:, :, 0:NMM],
                        func,
                        bias=bias[:, c:c + 1],
                    )
                    (sig_i, tg, sig_o)[role].append(dst)

                hs = []
                for ic in range(nchunks_h):
                    ct = apool.tile([128, NR], bf16, tag="c")
                    nc.vector.tensor_mul(ct[:], sig_i[ic][:], tg[ic][:])
                    if USE_TANH_POLY:
                        # tanh(c) ~= c*(A + B*c^2) on the DVE; c in (-1,1)
                        sq = apool.tile([128, NR], bf16, tag="sq")
                        nc.vector.tensor_mul(sq[:], ct[:], ct[:])
                        pq = apool.tile([128, NR], bf16, tag="pq")
                        nc.vector.tensor_scalar(
                            pq[:], sq[:], TP_B, TP_A,
                            op0=mybir.AluOpType.mult, op1=mybir.AluOpType.add)
                        tct = apool.tile([128, NR], bf16, tag="tc")
                        nc.vector.tensor_mul(tct[:], pq[:], ct[:])
                    else:
                        tct = apool.tile([128, NR], bf16, tag="tc")
                        nc.scalar.activation(tct[:], ct[:], AF.Tanh)
                    ht = hpool.tile([128, NR], bf16, tag="h")
                    nc.vector.tensor_mul(ht[:], sig_o[ic][:], tct[:])
                    hs.append(ht)
                return hs

            for j in range(n_tiles):
                b0 = j * NB
                xt = xpool.tile([128, NB, T], bf16)
                nc.sync.dma_start(xt[:], x_d[:, b0:b0 + NB, :])
                xt2 = xt[:].rearrange("p b t -> p (b t)")

                h1 = lstm_layer([xt2], [w1t[:]], b1, 2)
                h2 = lstm_layer([h[:] for h in h1], [w2t_k0[:], w2t_k1[:]], b2, 2)
                h3 = lstm_layer([h[:] for h in h2], [w3t_k0[:], w3t_k1[:]], b3, 1)
                h3t = h3[0]

                # fc1 (replicated M=21) + fc2 (elementwise * tiled weights, then
                # segmented reduce over t) per 336-row chunk
                for q in range(NR // NFC):
                    yps = ypool.tile([21, NFC], f32, tag="y")
                    nc.tensor.matmul(
                        yps[:],
                        fmat[:],
                        h3t[:, q * NFC:(q + 1) * NFC],
                        start=True,
                        stop=True,
                    )
                    pt = fpool.tile([21, NFC], f32, tag="pt")
                    nc.vector.tensor_mul(pt[:], yps[:], w2rep[:])
                    cb = b0 + q * (NFC // T)
                    nc.vector.tensor_reduce(
                        out_t[:, cb:cb + NFC // T],
                        pt[:].rearrange("p (b t) -> p b t", t=T),
                        axis=mybir.AxisListType.X,
                        op=mybir.AluOpType.add,
                    )

            nc.scalar.add(out_f[:], out_t[:], cst[:, 0:1])
            nc.sync.dma_start(out_d[:], out_f[:])

    return nc


def _legalize_pe_waits(nc):
    """This walrus build supports only ONE sync-wait command per engine
    instruction (setupSyncWait raises "Too many sync wait commands").  Hoist
    all but one wait onto NoOp instructions on the same engine queue just
    before the instruction — queues dispatch in order, so stalling at the
    nop is equivalent.
    """
    import bass_rust
    from concourse import mybir

    skip = (bass_rust.InstNoOp,)
    ctr = [0]

    def mk_nop(wait, engine):
        ctr[0] += 1
        n = bass_rust.InstNoOp(name=f"I-wfix-{ctr[0]}", ins=[], outs=[])
        n.engine = engine
        n.sync_info = bass_rust.SyncInfo(on_wait=[wait], on_update=[])
        return n

    for blk in nc.m.functions[0].blocks:
        out = []
        for inst in blk.instructions:
            si = inst.sync_info
            if (si is not None and len(si.on_wait) > 1
                    and not isinstance(inst, skip)):
                waits = list(si.on_wait)
                for w in waits[:-1]:
                    out.append(mk_nop(w, inst.engine))
                inst.sync_info = bass_rust.SyncInfo(
                    on_wait=[waits[-1]], on_update=list(si.on_update))
            out.append(inst)
        blk.instructions = out


def _prep_consts(W1_ih, b1_ih, b1_hh, W2_ih, b2_ih, b2_hh,
                 W3_ih, b3_ih, b3_hh, fc1_w, fc1_b, fc2_w, fc2_b):
    """Host-side layout prep of the small weights (shared across cores)."""
    import ml_dtypes
    bf = ml_dtypes.bfloat16

    def igo(w, bih, bhh, h):
        # pytorch gate order i,f,g,o; f unused
        wi, _, wg, wo = w[0:h], w[h:2 * h], w[2 * h:3 * h], w[3 * h:4 * h]
        b = (bih + bhh).astype(np.float32)
        bi, bg, bo = b[0:h], b[2 * h:3 * h], b[3 * h:4 * h]
        wt = np.concatenate([wi, wg, wo], axis=0).T.copy()     # (in, 3h)
        bb = np.concatenate([bi, bg, bo])                      # (3h,)
        # bias per chunk: [128, nchunks]
        bc = bb.reshape(-1, 128).T.copy()
        return np.ascontiguousarray(wt).astype(bf), np.ascontiguousarray(bc, np.float32)

    w1t, b1c = igo(W1_ih, b1_ih, b1_hh, H1)
    w2t, b2c = igo(W2_ih, b2_ih, b2_hh, H2)
    w3t, b3c = igo(W3_ih, b3_ih, b3_hh, H3)
    fmat = np.tile(fc1_w.reshape(I, 1), (1, T)).astype(bf)              # (128, 21)
    w2rep = np.tile(fc2_w, (1, NFC // T)).astype(np.float32)            # (21, 336)
    cst = (fc2_b + fc1_b[0] * fc2_w.sum(axis=1)).reshape(T, 1).astype(np.float32)
    return dict(w1t=w1t, w2t=w2t, w3t=w3t, b1=b1c, b2=b2c, b3=b3c,
                f=fmat, w2rep=w2rep, cst=cst)


def _make_in_maps(x, W1_ih, b1_ih, b1_hh, W2_ih, b2_ih, b2_hh,
                  W3_ih, b3_ih, b3_hh, fc1_w, fc1_b, fc2_w, fc2_b):
    import ml_dtypes
    bf = ml_dtypes.bfloat16

    consts = _prep_consts(W1_ih, b1_ih, b1_hh, W2_ih, b2_ih, b2_hh,
                          W3_ih, b3_ih, b3_hh, fc1_w, fc1_b, fc2_w, fc2_b)
    xb = np.asarray(x).astype(bf)                       # (B, I, T) bf16
    in_maps = []
    for c in range(NCORES):
        m = dict(consts)
        # per-core slice, transposed to [I, BS, T] so each DMA line is
        # contiguous per partition
        m["x"] = np.ascontiguousarray(xb[c * BS:(c + 1) * BS].transpose(1, 0, 2))
        in_maps.append(m)
    return in_maps


def kernel(x, W1_ih, b1_ih, b1_hh, W2_ih, b2_ih, b2_hh,
           W3_ih, b3_ih, b3_hh, fc1_w, fc1_b, fc2_w, fc2_b):
    from concourse.bass_utils import run_bass_kernel_spmd

    if "nc" not in _prog_cache:
        nc = _build_program(BS)
        _legalize_pe_waits(nc)   # HW-compile only; CoreSim can't sim the nops
        _prog_cache["nc"] = nc
    nc = _prog_cache["nc"]

    in_maps = _make_in_maps(x, W1_ih, b1_ih, b1_hh, W2_ih, b2_ih, b2_hh,
                            W3_ih, b3_ih, b3_hh, fc1_w, fc1_b, fc2_w, fc2_b)

    res = run_bass_kernel_spmd(nc, in_maps, list(range(NCORES)))
    outs = [r["out"] for r in res.results]          # each (21, BS)
    full = np.concatenate([o.T[:, None, :] for o in outs], axis=0)
    return full.astype(np.float32)
